# revision 26
# baseline (speedup 1.0000x reference)
"""Trainium2 Bass kernel for nn_CombinedLoss (Poisson + 3-way pairwise CLIP loss).

Strategy (8 NeuronCores, SPMD, one tiny AllGather):
  - Row-shard the batch: core c owns rows [c*512, (c+1)*512) of every tensor.
  - Similarity tiles are computed TRANSPOSED: for pair (a,b) the stationary
    matmul operand is the full feature b, host-transposed to [D, B] and cast
    to fp8/bf16 (a pure layout/dtype change -- all math stays on device); the
    moving operand is this core's own 512 rows of a, normalized on-device and
    PE-transposed.  out[j, i] = z_a[i] . b_raw[j].
  - With j on the PSUM partition axis, the 1/||b_j|| normalization folds into
    the exp's per-partition scale: exp(out * recipB_j / T).  Nobody ever
    normalizes (or even loads a row-major copy of) the full features.
  - recipB for all 4096 rows: each core computes squared norms of its OWN
    rows (DVE square-accum, nearly free) and a 4KB AllGather shares them.
  - Row sums of exp(sim/T): ones-matmul over the j-partition axis, PSUM-
    accumulated across all 32 j-tiles -> complete on device.
  - Column-sum partials: free via the exp activation's accum_out.
  - fp8 (e4m3) DoubleRow matmuls double PE throughput; fp32 accumulate.
  - Host does only the O(B) final combine: log of sums, means.
"""

import sys

import numpy as np

sys.path.insert(0, "/opt/trn_rl_repo")

P = 128
TEMPERATURE = 0.5
EPS_POISSON = 1e-8


class Cfg:
    def __init__(self, B=4096, D=1024, n_cores=8, fp8=True):
        self.B = B          # batch
        self.D = D          # feature dim
        self.n_cores = n_cores
        self.fp8 = fp8
        self.S = B // n_cores      # own rows per core
        self.MT = self.S // P      # own-row tiles (streaming free dim chunks)
        self.K = D // P            # contraction tiles
        self.JT = B // P           # stationary j tiles
        assert B % n_cores == 0 and self.S % P == 0 and D % P == 0
        if fp8:
            assert self.K % 2 == 0


def _patch_act_tables():
    """Make Bacc's act-table pass pick `natural_log_exp_and_others` for both
    Exp and Ln (they otherwise land in two different sets, and alternating
    Ln/Exp calls reload the 2.7us activation tables every tile)."""
    import functools

    import concourse.hw_specs as hw_specs

    if getattr(hw_specs, "_act_tables_patched", False):
        return
    orig = hw_specs.get_activation_tables

    @functools.cache
    def patched(module_arch):
        tabs = dict(orig(module_arch))
        names = list(tabs.keys())
        if "natural_log_exp_and_others" in tabs:
            combined = tabs["natural_log_exp_and_others"]
            for name in names:
                if name == "natural_log_exp_and_others":
                    break
                if tabs[name] & combined:
                    tabs[name] = tabs[name] - combined
        return tabs

    hw_specs.get_activation_tables = patched
    import concourse.bacc as bacc_mod

    if hasattr(bacc_mod, "get_activation_tables"):
        bacc_mod.get_activation_tables = patched
    hw_specs._act_tables_patched = True


def build_bass(cfg: Cfg):
    """Build the single-core Bass program (same program for all SPMD cores)."""
    import concourse.bacc as bacc
    import concourse.bass as bass
    import concourse.mybir as mybir
    import concourse.tile as tile
    from concourse.masks import make_identity

    _patch_act_tables()

    f32 = mybir.dt.float32
    bf16 = mybir.dt.bfloat16
    fp8 = mybir.dt.float8e4
    AF = mybir.ActivationFunctionType
    ALU = mybir.AluOpType
    ts = bass.ts

    B, D, K, MT, JT, S = cfg.B, cfg.D, cfg.K, cfg.MT, cfg.JT, cfg.S
    NC = cfg.n_cores
    mm_dt = fp8 if cfg.fp8 else bf16

    nc = bacc.Bacc(
        "TRN2",
        target_bir_lowering=False,
        debug=False,
        enable_asserts=False,
        num_devices=NC,
    )

    # ---- IO ----
    # fT2/fT3: full features 2,3 transposed [D, B], raw (unnormalized)
    fT2 = nc.dram_tensor("fT2", [D, B], mm_dt, kind="ExternalInput").ap()
    fT3 = nc.dram_tensor("fT3", [D, B], mm_dt, kind="ExternalInput").ap()
    # own row slices, bf16
    f1o = nc.dram_tensor("f1_own", [S, D], bf16, kind="ExternalInput").ap()
    f2o = nc.dram_tensor("f2_own", [S, D], bf16, kind="ExternalInput").ap()
    f3o = nc.dram_tensor("f3_own", [S, D], bf16, kind="ExternalInput").ap()
    inp = nc.dram_tensor("inp_own", [S, D], bf16, kind="ExternalInput").ap()
    tgt = nc.dram_tensor("tgt_own", [S, D], bf16, kind="ExternalInput").ap()

    rowsum_d = nc.dram_tensor("rowsum", [1, 3 * S], f32, kind="ExternalOutput").ap()
    colparts_d = nc.dram_tensor("colparts", [P, 3 * JT], f32, kind="ExternalOutput").ap()
    nsq_d = nc.dram_tensor("nsq_own", [P, 3 * MT], f32, kind="ExternalOutput").ap()
    dots_d = nc.dram_tensor("dots_own", [P, 3 * MT], f32, kind="ExternalOutput").ap()
    poi_d = nc.dram_tensor("poi", [P, 2 * MT], f32, kind="ExternalOutput").ap()

    own_dram = [f1o, f2o, f3o]
    fT_dram = [fT2, fT3]

    with tile.TileContext(nc) as tc:
        with (
            tc.tile_pool(name="const", bufs=1) as const_pool,
            tc.tile_pool(name="persist", bufs=1) as persist,
            tc.tile_pool(name="own", bufs=1) as ownp,
            tc.tile_pool(name="stage", bufs=2) as stage,
            tc.tile_pool(name="junk", bufs=2) as junkp,
            tc.tile_pool(name="exps", bufs=6) as expp,
            tc.tile_pool(name="small", bufs=8) as smallp,
            tc.tile_pool(name="dscr", bufs=1, space="DRAM") as dramp,
            tc.tile_pool(name="ps_s", bufs=2, space="PSUM") as ps_s,
            tc.tile_pool(name="ps_t", bufs=1, space="PSUM") as ps_t,
            tc.tile_pool(name="ps_r", bufs=3, space="PSUM") as ps_r,
        ):
            identity = const_pool.tile([P, P], bf16)
            make_identity(nc, identity)
            ones = const_pool.tile([P, 1], bf16)
            nc.vector.memset(ones, 1.0)
            ones_f8 = const_pool.tile([P, 2, 16], fp8)
            nc.vector.memset(ones_f8, 1.0)
            eps_bias = const_pool.tile([P, 1], f32)
            nc.vector.memset(eps_bias, EPS_POISSON)

            # persistent state
            fT_sb = [persist.tile([P, K, B], mm_dt, name=f"fT_sb{i}") for i in range(2)]
            zT = [persist.tile([P, K, S], mm_dt, name=f"zT{i}") for i in range(2)]
            nsq_own = persist.tile([P, 3 * MT], f32)
            dots_own = persist.tile([P, 3 * MT], f32)
            poi = persist.tile([P, 2 * MT], f32)
            colparts = persist.tile([P, 3 * JT], f32)
            rowsum_sb = persist.tile([1, 3 * S], f32)
            scaleB = [persist.tile([P, JT], f32, name=f"scaleB{i}") for i in range(2)]
            # sim-stash for tiles computed before the gathered norms arrive
            N_STASH_F = min(6, JT)    # fused groups stashed (2 tiles each)
            assert N_STASH_F % 2 == 0 and JT % 2 == 0
            st_dt = fp8
            stash12 = persist.tile([P, JT, S], st_dt)
            stashF = persist.tile([P, N_STASH_F, 2, S], fp8)
            # exp-tile stores: contiguous so row-sum matmuls can consume
            # adjacent jt pairs in one fp8 DoubleRow matmul
            es12 = persist.tile([P, JT, S], fp8)
            esF = persist.tile([P, JT, 2, S], fp8)

            own_rf = {}  # (fi, t) -> own bf16 row tile

            def rsqrt_act(dst, src, n, tag):
                # dst[:, :n] = 1/sqrt(src) = exp(-0.5*ln(src))
                l = smallp.tile([P, n], f32, tag=tag)
                nc.scalar.activation(l, src, AF.Ln)
                nc.scalar.activation(dst, l, AF.Exp, scale=-0.5)

            # ---- Phase A1: own loads + squared norms -> collective, ASAP ----
            # sync queue: f1o (zT critical path), f3o+f2o (collective), fT2.
            for fi in (0, 2, 1):
                for t in range(MT):
                    rf = ownp.tile([P, D], bf16, tag=f"own{fi}_{t}")
                    nc.sync.dma_start(rf, own_dram[fi][ts(t, P), :])
                    own_rf[(fi, t)] = rf
            for t in range(MT):  # f1 squares on ACT (first: feeds zT[0])
                jt_ = junkp.tile([P, D], bf16, tag="junk16")
                nc.scalar.activation(
                    jt_, own_rf[(0, t)], AF.Square,
                    accum_out=nsq_own[:, t : t + 1],
                )
            recip_own = smallp.tile([P, 2 * MT], f32, tag="recip_own")
            rsqrt_act(recip_own[:, :MT], nsq_own[:, :MT], MT, tag="ln_own1")
            for t in range(MT):  # f2 squares on DVE (collective input)
                jt_ = junkp.tile([P, D], bf16, tag="junk16")
                nc.vector.scalar_tensor_tensor(
                    out=jt_, in0=own_rf[(1, t)], scalar=1.0, in1=own_rf[(1, t)],
                    op0=ALU.mult, op1=ALU.mult,
                    accum_out=nsq_own[:, MT + t : MT + t + 1],
                )
            for t in range(MT):  # f3 squares on ACT (collective input)
                jt_ = junkp.tile([P, D], bf16, tag="junk16")
                nc.scalar.activation(
                    jt_, own_rf[(2, t)], AF.Square,
                    accum_out=nsq_own[:, 2 * MT + t : 2 * MT + t + 1],
                )
            cc_in = dramp.tile([P, 2 * MT], f32, name="cc_in")
            cc_out = dramp.tile([NC * P, 2 * MT], f32, name="cc_out")
            nc.gpsimd.dma_start(cc_in[:], nsq_own[:, MT : 3 * MT])
            nc.gpsimd.collective_compute(
                "AllGather", mybir.AluOpType.bypass,
                replica_groups=[list(range(NC))],
                ins=[cc_in.opt()], outs=[cc_out.opt()],
            )
            for k in range(K):  # fT2 on sync queue
                nc.sync.dma_start(fT_sb[0][:, k, :], fT_dram[0][ts(k, P), :])
            for k in range(K):  # fT3 on gpsimd queue
                nc.gpsimd.dma_start(fT_sb[1][:, k, :], fT_dram[1][ts(k, P), :])

            # ---- Phase A2: normalize + PE-transpose own rows into zT ----
            def make_zT(a, t, copy_eng):
                # normalize on ACT (copy with per-partition scale)
                zrow = stage.tile([P, D], bf16, tag="zrow")
                nc.scalar.activation(
                    zrow, own_rf[(a, t)], AF.Copy,
                    scale=recip_own[:, a * MT + t : a * MT + t + 1],
                )
                tps = ps_t.tile([P, K * P], bf16, tag="tps")
                for k in range(K):
                    nc.tensor.transpose(tps[:, ts(k, P)], zrow[:, ts(k, P)], identity)
                copy_eng.tensor_copy(
                    out=zT[a][:, :, ts(t, P)],
                    in_=tps[:].rearrange("p (k c) -> p k c", k=K),
                )

            for t in range(MT):
                make_zT(0, t, nc.vector)
            rsqrt_act(recip_own[:, MT:], nsq_own[:, MT : 2 * MT], MT, tag="ln_own2")
            for t in range(MT):
                make_zT(1, t, nc.vector)

            for pi, (ia, ib) in enumerate(((0, 1), (0, 2), (1, 2))):
                for t in range(MT):
                    jt_ = junkp.tile([P, D], bf16, tag="junk16")
                    nc.vector.scalar_tensor_tensor(
                        out=jt_, in0=own_rf[(ia, t)], scalar=1.0, in1=own_rf[(ib, t)],
                        op0=ALU.mult, op1=ALU.mult,
                        accum_out=dots_own[:, pi * MT + t : pi * MT + t + 1],
                    )

            # ---- Phase B: main matmuls ----
            # orientation: out[j, i] = fT_b[:, j] . zT_a[:, i]; stationary fT_b.
            def mm_group(b, a, ps, jt):
                if cfg.fp8:
                    for kk in range(0, K, 2):
                        nc.tensor.matmul(
                            ps,
                            fT_sb[b][:, kk : kk + 2, ts(jt, P)],
                            zT[a][:, kk : kk + 2, :],
                            start=(kk == 0), stop=(kk == K - 2),
                            perf_mode=mybir.MatmulPerfMode.DoubleRow,
                            skip_group_check=True,
                        )
                else:
                    for k in range(K):
                        nc.tensor.matmul(
                            ps,
                            fT_sb[b][:, k, ts(jt, P)],
                            zT[a][:, k, :],
                            start=(k == 0), stop=(k == K - 1),
                            skip_group_check=True,
                        )

            def mm_group_fused(ps13, ps23, jt):
                # pairs (f1,f3) and (f2,f3) share stationary fT3 -> one
                # LDWEIGHTS per (jt, kk) serves two matmuls.
                if cfg.fp8:
                    for kk in range(0, K, 2):
                        w = fT_sb[1][:, kk : kk + 2, ts(jt, P)]
                        for ps, a in ((ps13, 0), (ps23, 1)):
                            nc.tensor.matmul(
                                ps, w, zT[a][:, kk : kk + 2, :],
                                start=(kk == 0), stop=(kk == K - 2),
                                perf_mode=mybir.MatmulPerfMode.DoubleRow,
                                skip_group_check=True,
                            )
                else:
                    for k in range(K):
                        w = fT_sb[1][:, k, ts(jt, P)]
                        for ps, a in ((ps13, 0), (ps23, 1)):
                            nc.tensor.matmul(
                                ps, w, zT[a][:, k, :],
                                start=(k == 0), stop=(k == K - 1),
                                skip_group_check=True,
                            )

            def colsum(pi, jt, src):
                slot = pi * JT + jt
                nc.vector.tensor_reduce(
                    out=colparts[:, slot : slot + 1], in_=src,
                    axis=mybir.AxisListType.X, op=ALU.add,
                )

            def exp12(jt, src):
                # pair (f1,f2): es12[:, jt] = exp(src * recipB2_j / T)
                nc.scalar.activation(
                    es12[:, jt, :], src, AF.Exp, scale=scaleB[0][:, jt : jt + 1]
                )
                colsum(0, jt, es12[:, jt, :])

            def expF(jt, src2):
                # fused pairs share the j-tile and thus the scale: one exp
                # over both [P, S] halves (src2 is [P, 2*S]-shaped)
                nc.scalar.activation(
                    esF[:, jt, :, :].rearrange("p q s -> p (q s)"), src2,
                    AF.Exp, scale=scaleB[1][:, jt : jt + 1],
                )
                colsum(1, jt, esF[:, jt, 0, :])
                colsum(2, jt, esF[:, jt, 1, :])

            def build_scaleB():
                # gathered norms -> per-partition exp scales (collective
                # result); emitted mid-M2 so the in-order ACT stream does
                # not block earlier stash copies on the collective.
                nsqB = smallp.tile([P, NC, 2 * MT], f32, tag="nsqB")
                nc.sync.dma_start(
                    nsqB, cc_out[:].rearrange("(r p) m -> p r m", r=NC)
                )
                for b in range(2):
                    lnB = smallp.tile([P, JT], f32, tag=f"lnB{b}")
                    nc.scalar.activation(
                        lnB[:].rearrange("p (r m) -> p r m", r=NC),
                        nsqB[:, :, b * MT : (b + 1) * MT],
                        AF.Ln,
                    )
                    recipB = smallp.tile([P, JT], f32, tag=f"recipB{b}")
                    nc.scalar.activation(recipB, lnB, AF.Exp, scale=-0.5)
                    nc.vector.tensor_scalar_mul(
                        scaleB[b], recipB, 1.0 / TEMPERATURE
                    )

            # Row-sum accumulators: one PSUM bank per pair; adjacent exp-store
            # jt pairs are consumed by fp8 DoubleRow ones-matmuls (K=256).
            NQ = JT // 2
            rs_ps = {pi: ps_r.tile([1, S], f32, tag="ps_r", name=f"rs{pi}")
                     for pi in range(3)}
            rs_count = {0: 0, 1: 0, 2: 0}

            def emit_rowsum_pair(pi, q):
                if pi == 0:
                    rhs = es12[:, 2 * q : 2 * q + 2, :]
                else:
                    rhs = esF[:, 2 * q : 2 * q + 2, pi - 1, :]
                n = rs_count[pi]
                nc.tensor.matmul(rs_ps[pi], ones_f8[:, :, 0:1], rhs,
                                 start=(n == 0), stop=(n == NQ - 1),
                                 perf_mode=mybir.MatmulPerfMode.DoubleRow,
                                 skip_group_check=True)
                rs_count[pi] = n + 1

            # stash items: exp order once scaleB lands. pair12 items exp one
            # [P,S] tile; fused items exp a whole [P,2S] group.
            stash_items = [("12", jt) for jt in range(JT)]
            stash_items += [("F", jt) for jt in range(min(N_STASH_F, JT))]
            CHUNK = 2
            rs_ready = []  # (pi, q) rowsum pairs ready to emit next group

            def exp_stash_item(item):
                kind, jt_s = item
                if kind == "12":
                    exp12(jt_s, stash12[:, jt_s, :])
                    if jt_s % 2 == 1:
                        rs_ready.append((0, jt_s // 2))
                else:
                    expF(jt_s, stashF[:, jt_s, :, :].rearrange("p q s -> p (q s)"))
                    if jt_s % 2 == 1:
                        rs_ready.append((1, jt_s // 2))
                        rs_ready.append((2, jt_s // 2))

            # M1: pair (f1,f2): all groups stashed (gathered norms not ready
            # yet); two jt groups share one 2-bank PSUM tile and one copy.
            for jt in range(0, JT, 2):
                ps2 = ps_s.tile([P, 2 * S], f32, tag="ps2")
                mm_group(0, 0, ps2[:, 0:S], jt)
                mm_group(0, 0, ps2[:, S : 2 * S], jt + 1)
                nc.scalar.activation(stash12[:, jt : jt + 2, :], ps2, AF.Copy)

            # M2: fused pairs (f1,f3), (f2,f3): first N_STASH_F groups
            # stashed, rest exp'd directly from PSUM; stash-exps and
            # row-sums spread across the direct groups with a lag.
            for jt in range(JT):
                ps2 = ps_s.tile([P, 2 * S], f32, tag="ps2")
                mm_group_fused(ps2[:, 0:S], ps2[:, S : 2 * S], jt)
                for pi_s, q_s in rs_ready:
                    emit_rowsum_pair(pi_s, q_s)
                rs_ready = []
                if jt < N_STASH_F:
                    nc.scalar.activation(stashF[:, jt, :, :].rearrange("p q s -> p (q s)"), ps2, AF.Copy)
                else:
                    if jt == N_STASH_F:
                        build_scaleB()
                    expF(jt, ps2)
                    if jt % 2 == 1 and (jt - 1) >= N_STASH_F:
                        rs_ready.append((1, jt // 2))
                        rs_ready.append((2, jt // 2))
                    for _ in range(CHUNK):
                        if stash_items:
                            exp_stash_item(stash_items.pop(0))
            if N_STASH_F >= JT:
                build_scaleB()

            # M3: drain whatever is left
            for pi_s, q_s in rs_ready:
                emit_rowsum_pair(pi_s, q_s)
            rs_ready = []
            for item in stash_items:
                exp_stash_item(item)
            stash_items = []
            for pi_s, q_s in rs_ready:
                emit_rowsum_pair(pi_s, q_s)
            assert rs_count[0] == rs_count[1] == rs_count[2] == NQ
            for pi in range(3):
                nc.any.tensor_copy(
                    out=rowsum_sb[:, pi * S : (pi + 1) * S], in_=rs_ps[pi])

            # ---- Phase C: poisson tail ----
            for t in range(MT):
                it = stage.tile([P, D], bf16, tag="it")
                tt = stage.tile([P, D], bf16, tag="tt")
                nc.sync.dma_start(it, inp[ts(t, P), :])
                nc.sync.dma_start(tt, tgt[ts(t, P), :])
                lg = stage.tile([P, D], f32, tag="lg")
                nc.scalar.activation(lg, it, AF.Ln, bias=eps_bias[:, :])
                jt_ = junkp.tile([P, D], bf16, tag="junk16")
                nc.vector.scalar_tensor_tensor(
                    out=jt_, in0=tt, scalar=1.0, in1=lg,
                    op0=ALU.mult, op1=ALU.mult,
                    accum_out=poi[:, MT + t : MT + t + 1],
                )
                jt2 = junkp.tile([P, D], bf16, tag="junk16")
                nc.vector.tensor_scalar(
                    out=jt2, in0=it, scalar1=1.0, scalar2=0.0, op0=ALU.mult,
                    op1=ALU.add, accum_out=poi[:, t : t + 1],
                )

            # ---- outputs ----
            nc.gpsimd.dma_start(rowsum_d, rowsum_sb)
            nc.gpsimd.dma_start(colparts_d, colparts)
            nc.gpsimd.dma_start(nsq_d, nsq_own)
            nc.gpsimd.dma_start(dots_d, dots_own)
            nc.gpsimd.dma_start(poi_d, poi)

    nc.compile()
    return nc


def make_in_maps(cfg: Cfg, inputs, targets, feature1, feature2, feature3):
    import ml_dtypes

    bf16 = ml_dtypes.bfloat16
    mm_np = ml_dtypes.float8_e4m3 if cfg.fp8 else bf16
    ac = np.ascontiguousarray

    # shared across cores: full transposed raw features (layout+dtype only)
    fT2 = ac(feature2.T).astype(mm_np)
    fT3 = ac(feature3.T).astype(mm_np)
    f1b = feature1.astype(bf16)
    f2b = feature2.astype(bf16)
    f3b = feature3.astype(bf16)
    inb = inputs.astype(bf16)
    tgb = targets.astype(bf16)

    maps = []
    for c in range(cfg.n_cores):
        sl = slice(c * cfg.S, (c + 1) * cfg.S)
        maps.append({
            "fT2": fT2,
            "fT3": fT3,
            "f1_own": ac(f1b[sl]),
            "f2_own": ac(f2b[sl]),
            "f3_own": ac(f3b[sl]),
            "inp_own": ac(inb[sl]),
            "tgt_own": ac(tgb[sl]),
        })
    return maps


def combine_results(cfg: Cfg, per_core):
    """per_core: list of dicts with rowsum/colparts/nsq_own/dots_own/poi."""
    B, MT, JT, S = cfg.B, cfg.MT, cfg.JT, cfg.S
    nsq = np.zeros((3, B), np.float64)
    dots = np.zeros((3, B), np.float64)
    rowsum = np.zeros((3, B), np.float64)
    colsum = np.zeros((3, B), np.float64)
    poi_in = 0.0
    poi_tl = 0.0
    for c, r in enumerate(per_core):
        rs = np.asarray(r["rowsum"], np.float64)[0]    # [3*S]
        cp = np.asarray(r["colparts"], np.float64)     # [128, 3*JT]
        nq = np.asarray(r["nsq_own"], np.float64)      # [128, 3*MT]
        dt_ = np.asarray(r["dots_own"], np.float64)
        po = np.asarray(r["poi"], np.float64)          # [128, 2*MT]
        for fi in range(3):
            for t in range(MT):
                nsq[fi, c * S + t * P : c * S + (t + 1) * P] = nq[:, fi * MT + t]
        for pi in range(3):
            rowsum[pi, c * S : (c + 1) * S] = rs[pi * S : (pi + 1) * S]
            for t in range(MT):
                rows = slice(c * S + t * P, c * S + (t + 1) * P)
                dots[pi, rows] = dt_[:, pi * MT + t]
            for jt in range(JT):
                colsum[pi, jt * P : (jt + 1) * P] += cp[:, pi * JT + jt]
        poi_in += po[:, :MT].sum()
        poi_tl += po[:, MT:].sum()

    na = np.sqrt(nsq)  # [3, B]
    pairs = ((0, 1), (0, 2), (1, 2))
    closs = 0.0
    for pi, (ia, ib) in enumerate(pairs):
        simdiag = dots[pi] / (na[ia] * na[ib])
        loss_i = np.mean(np.log(rowsum[pi]) - simdiag / TEMPERATURE)
        loss_j = np.mean(np.log(colsum[pi]) - simdiag / TEMPERATURE)
        closs += 0.5 * (loss_i + loss_j)
    closs /= 3.0
    p_loss = (poi_in - poi_tl) / (cfg.B * cfg.D)
    total = p_loss + closs
    return (
        np.float32(total),
        np.float32(p_loss),
        np.float32(closs),
    )


_CACHE = {}


def _get_compiled(cfg: Cfg):
    key = (cfg.B, cfg.D, cfg.n_cores, cfg.fp8)
    if key not in _CACHE:
        _CACHE[key] = build_bass(cfg)
    return _CACHE[key]


def kernel(inputs, targets, feature1, feature2, feature3):
    from concourse.bass_utils import run_bass_kernel_spmd

    cfg = Cfg(B=inputs.shape[0], D=inputs.shape[1], n_cores=8)
    nc = _get_compiled(cfg)
    in_maps = make_in_maps(cfg, inputs, targets, feature1, feature2, feature3)
    res = run_bass_kernel_spmd(nc, in_maps, core_ids=list(range(cfg.n_cores)))
    return combine_results(cfg, res.results)


if __name__ == "__main__":
    rng = np.random.default_rng(0)
    B, D = 4096, 1024
    ins = {
        "inputs": rng.random((B, D), np.float32),
        "targets": rng.random((B, D), np.float32),
        "feature1": rng.standard_normal((B, D)).astype(np.float32),
        "feature2": rng.standard_normal((B, D)).astype(np.float32),
        "feature3": rng.standard_normal((B, D)).astype(np.float32),
    }
    out = kernel(**ins)
    print(out)
